# revision 1
# baseline (speedup 1.0000x reference)
"""GPT forward pass (B=2,T=1024,C=768,H=12,L=6,V=32000) on 8 TRN2 NeuronCores.

Sharding: context/token parallel. Token blocks of 128; batch bt=r//4, local
rank lr=r%4; core r owns query blocks {lr, 7-lr} of its batch (balanced causal
work). Per layer the LN1 activations x-hat (bf16) are all-gathered within each
4-core batch group as TWO half-column collectives (first the early global
blocks 0..3, then 4..7) so K/V recompute + early-block attention overlap the
second gather. K/V for the full sequence are recomputed locally from the
gathered x-hat (PE matmul cost is output-columns only, so recompute beats
shipping K/V). LM head is token-parallel (each core: own 256 tokens x full
vocab) so no final collective is needed.

Attention computes scores transposed, S^T[k,q] = K Q^T, over a rank-uniform
fixed set of kv blocks (4 for the early query block, 8 for the late one);
causality and the rank-varying diagonal live in a host-supplied 0/1 mask
multiplied into exp(S^T). No row-max is needed (scores are O(0.1)); the
softmax denominator falls out of a ones-column appended to V, and
normalization is a rank-1 broadcast matmul. No transposes anywhere.

Activations stay feature-major [C_part, token_free]; LN stats/broadcasts via
rank-1 bf16 PE matmuls.
"""

import sys

for _p in (
    "/opt/trn_rl_repo",
    "/opt/pypackages",
    "/root/.axon_site",
    "/root/.axon_site/_ro/trn_rl_repo",
    "/root/.axon_site/_ro/pypackages",
):
    if _p not in sys.path:
        sys.path.append(_p)

import numpy as np
import ml_dtypes

import concourse.bass as bass
import concourse.mybir as mybir
import concourse.tile as tile
from concourse import bacc
from concourse.bass_utils import run_bass_kernel_spmd

BF16 = mybir.dt.bfloat16
F32 = mybir.dt.float32
AF = mybir.ActivationFunctionType
OP = mybir.AluOpType

B, T, C, H, L, V = 2, 1024, 768, 12, 6, 32000
HS, P = 64, 128
NCORES = 8
FT = C // P  # 6 feature tiles
F4 = 4 * C // P  # 24 ffn tiles
TB = 256  # tokens per core
NB = T // P  # 8 blocks per batch sequence
VC2 = 1024  # vocab chunk for the head
EPS = 1e-5
SCALE = C ** -0.5
QW = (4, 8)  # rank-uniform kv-block widths for the two owned query blocks
# global block gb -> column offset in shard-ordered full-sequence buffers
# (shard s contributes its blocks s and 7-s at column s*TB and s*TB+P)
COL = [0, 256, 512, 768, 896, 640, 384, 128]
SHX2 = FT * P * P  # 98304 bf16 elements per half-shard


def _blocks_of(rank):
    lr = rank % 4
    return [lr, 7 - lr]


def build(n_layers=L, attn=True, head=True):
    nc = bacc.Bacc("TRN2", target_bir_lowering=False, debug=False,
                   num_devices=NCORES)

    x0_d = nc.dram_tensor("x0", [P, FT, TB], BF16, kind="ExternalInput")
    wqkv_d = nc.dram_tensor("wqkv", [L, FT, P, 3 * C], BF16, kind="ExternalInput")
    wp_d = nc.dram_tensor("wp", [L, FT, P, C], BF16, kind="ExternalInput")
    w1_d = nc.dram_tensor("w1", [L, FT, P, 4 * C], BF16, kind="ExternalInput")
    w2_d = nc.dram_tensor("w2", [L, F4, P, C], BF16, kind="ExternalInput")
    wh_d = nc.dram_tensor("wh", [FT, P, V], BF16, kind="ExternalInput")
    lng_d = nc.dram_tensor("lng", [2 * L + 1, C], BF16, kind="ExternalInput")
    lnb_d = nc.dram_tensor("lnb", [2 * L + 1, C], F32, kind="ExternalInput")
    bp_d = nc.dram_tensor("bp", [L, C], F32, kind="ExternalInput")
    b1_d = nc.dram_tensor("b1", [L, 4 * C], F32, kind="ExternalInput")
    b2_d = nc.dram_tensor("b2", [L, C], F32, kind="ExternalInput")
    msk_d = nc.dram_tensor("msk", [P, 8 * P], BF16, kind="ExternalInput")
    out_d = nc.dram_tensor("out", [2, P, V], BF16, kind="ExternalOutput")

    with tile.TileContext(nc) as tc:
        with (
            tc.tile_pool(name="const", bufs=1) as cp,
            tc.tile_pool(name="act", bufs=1) as ap,
            tc.tile_pool(name="rows", bufs=2) as rp,
            tc.tile_pool(name="dram", bufs=1, space="DRAM") as dp,
        ):
            # ---- constants ----
            ones_col_bf = cp.tile([P, 1], BF16, name="ones_col_bf")
            nc.vector.memset(ones_col_bf[:], 1.0)
            ones_row_bf = cp.tile([1, P], BF16, name="ones_row_bf")
            nc.vector.memset(ones_row_bf[:], 1.0)
            eps_c = cp.tile([1, 1], F32, name="eps_c")
            nc.vector.memset(eps_c[:], EPS)
            msk = cp.tile([P, 8 * P], BF16, name="msk")
            nc.scalar.dma_start(msk[:], msk_d.ap())

            with tc.tile_pool(name="psum", bufs=1, space="PSUM") as pp:

                def ln_params(i):
                    g_row = rp.tile([1, C], BF16, tag="grow")
                    nc.scalar.dma_start(g_row[:], lng_d.ap()[i : i + 1, :])
                    b_col = rp.tile([P, FT], F32, tag="bcol")
                    nc.scalar.dma_start(
                        b_col[:], lnb_d.ap()[i].rearrange("(f p) -> p f", p=P)
                    )
                    return g_row, b_col

                def layernorm(i, x_src, out_tag="xh"):
                    """x_src: [P, FT, TB] bf16 -> new tile [P, FT, TB] bf16."""
                    g_row, b_col = ln_params(i)
                    st1 = pp.tile([1, TB], F32, tag="s", bufs=2)
                    st2 = pp.tile([1, TB], F32, tag="s", bufs=2)
                    for f in range(FT):
                        sq = ap.tile([P, TB], BF16, tag="sq", bufs=2)
                        nc.scalar.square(sq[:], x_src[:, f, :])
                        nc.tensor.matmul(st1[:1, :], ones_col_bf[:],
                                         x_src[:, f, :],
                                         start=(f == 0), stop=(f == FT - 1))
                        nc.tensor.matmul(st2[:1, :], ones_col_bf[:], sq[:],
                                         start=(f == 0), stop=(f == FT - 1))
                    rinv, mean, m2, var, std, nmr = (
                        rp.tile([1, TB], F32, tag=t, name=t)[:1, :]
                        for t in ("rinv", "mean", "m2", "var", "std", "nmr")
                    )
                    nc.vector.tensor_scalar_mul(mean, st1[:1, :], 1.0 / C)
                    nc.vector.tensor_tensor(m2, mean, mean, op=OP.mult)
                    nc.vector.scalar_tensor_tensor(
                        var, in0=st2[:1, :], scalar=1.0 / C, in1=m2,
                        op0=OP.mult, op1=OP.subtract,
                    )
                    nc.scalar.activation(std, var, AF.Sqrt, bias=eps_c[:1, :1])
                    nc.vector.reciprocal(rinv, std)
                    nc.vector.scalar_tensor_tensor(
                        nmr, in0=mean, scalar=-1.0, in1=rinv,
                        op0=OP.mult, op1=OP.mult,
                    )
                    rinv_b = rp.tile([1, TB], BF16, tag="rinvb", name="rinvb")
                    nc.vector.tensor_copy(rinv_b[:1, :], rinv)
                    nmr_b = rp.tile([1, TB], BF16, tag="nmrb", name="nmrb")
                    nc.vector.tensor_copy(nmr_b[:1, :], nmr)
                    out = ap.tile([P, FT, TB], BF16, tag=out_tag, bufs=1)
                    for f in range(FT):
                        # gs and aa share one psum tile (one slot per f) so
                        # consecutive f iterations pipeline with bufs=2
                        ga = pp.tile([P, 2 * TB], F32, tag="g", bufs=2)
                        nc.tensor.matmul(ga[:, 0:TB],
                                         g_row[:1, f * P : (f + 1) * P],
                                         rinv_b[:1, :], start=True, stop=True)
                        nc.tensor.matmul(ga[:, TB : 2 * TB],
                                         g_row[:1, f * P : (f + 1) * P],
                                         nmr_b[:1, :], start=True, stop=True)
                        t1 = ap.tile([P, TB], BF16, tag="t1", bufs=2)
                        nc.vector.tensor_tensor(t1[:], x_src[:, f, :],
                                                ga[:, 0:TB], op=OP.mult)
                        nc.vector.scalar_tensor_tensor(
                            out[:, f, :], in0=t1[:], scalar=b_col[:, f : f + 1],
                            in1=ga[:, TB : 2 * TB], op0=OP.add, op1=OP.add,
                        )
                    return out

                # ---- embedding: host supplies feature-major bf16 tok+pos ----
                x_cur = ap.tile([P, FT, TB], BF16, tag="x", bufs=2)
                nc.sync.dma_start(x_cur[:], x0_d.ap())

                # ---- internal DRAM for the split x-hat all-gather ----
                ag_in = [dp.tile([1, SHX2], BF16, name=f"ag_in{i}")
                         for i in range(2)]
                ag_out = [dp.tile([4, SHX2], BF16, name=f"ag_out{i}")
                          for i in range(2)]

                # persistent V (token-major; 65th column stays 1.0 so the
                # softmax denominator falls out of the PV matmul)
                v_sb = ap.tile([P, NB, H, HS + 1], BF16, tag="vsb", bufs=1)
                nc.vector.memset(v_sb[:], 1.0)

                with tc.tile_pool(name="wts", bufs=1) as wp_pool:
                    for l in range(n_layers):
                        # wqkv prefetch: double-buffered + emitted before LN1
                        # so it lands during the previous layer's tail and Q
                        # can overlap the collectives
                        wqkv_t = wp_pool.tile([P, FT, 3 * C], BF16,
                                              tag="wqkv", bufs=2)
                        nc.scalar.dma_start(
                            wqkv_t[:], wqkv_d.ap()[l].rearrange("f p m -> p f m"))

                        xh = layernorm(2 * l, x_cur)

                        # -- split all-gather: half 0 = global blocks 0..3 --
                        # (launched before the weight DMAs so the in-order
                        # DMA queue doesn't delay the collective staging)
                        for hf in range(2):
                            nc.sync.dma_start(
                                ag_in[hf][0].rearrange("(f p t) -> p f t",
                                                       p=P, t=P),
                                xh[:, :, hf * P : (hf + 1) * P],
                            )
                            nc.gpsimd.collective_compute(
                                "AllGather", OP.bypass,
                                replica_groups=[[0, 1, 2, 3], [4, 5, 6, 7]],
                                ins=[ag_in[hf][:].opt()],
                                outs=[ag_out[hf][:].opt()],
                            )

                        # -- bias rows (Act queue, overlap the collectives) --
                        bpc = rp.tile([P, FT], F32, tag="bpc")
                        nc.scalar.dma_start(
                            bpc[:], bp_d.ap()[l].rearrange("(f p) -> p f", p=P))
                        b1c = rp.tile([P, F4], F32, tag="b1c")
                        nc.scalar.dma_start(
                            b1c[:], b1_d.ap()[l].rearrange("(f p) -> p f", p=P))
                        b2c = rp.tile([P, FT], F32, tag="b2c")
                        nc.scalar.dma_start(
                            b2c[:], b2_d.ap()[l].rearrange("(f p) -> p f", p=P))

                        # -- Q for own tokens (overlaps the collectives) --
                        q_sb = ap.tile([P, FT, TB], BF16, tag="q")
                        for o in range(FT):
                            ps = pp.tile([P, TB], F32, tag="g", bufs=2)
                            for f in range(FT):
                                nc.tensor.matmul(
                                    ps[:], wqkv_t[:, f, o * P : (o + 1) * P],
                                    xh[:, f, :], start=(f == 0),
                                    stop=(f == FT - 1),
                                )
                            nc.scalar.copy(q_sb[:, o, :], ps[:])

                        kf = ap.tile([P, FT, T], BF16, tag="kf")
                        xf = ap.tile([P, FT, T], BF16, tag="xf")
                        o_fm = ap.tile([P, FT, TB], BF16, tag="ofm")

                        def kv_half(hf):
                            """K/V for the 4 global blocks of half hf from the
                            gathered x-hat."""
                            for s in range(4):
                                c0 = s * TB + hf * P
                                nc.sync.dma_start(
                                    xf[:, :, c0 : c0 + P],
                                    ag_out[hf][s].rearrange(
                                        "(f p t) -> p f t", p=P, t=P),
                                )
                            for s in range(4):
                                c0 = s * TB + hf * P
                                for o in range(FT):
                                    ps = pp.tile([P, P], F32, tag="g", bufs=2)
                                    for f in range(FT):
                                        nc.tensor.matmul(
                                            ps[:],
                                            wqkv_t[:, f,
                                                   C + o * P : C + (o + 1) * P],
                                            xf[:, f, c0 : c0 + P],
                                            start=(f == 0), stop=(f == FT - 1),
                                        )
                                    nc.vector.tensor_copy(
                                        kf[:, o, c0 : c0 + P], ps[:])
                                vb = c0 // P
                                ps = pp.tile([P, H, HS], F32, tag="s", bufs=2)
                                # matmul out must stay within one 2KB PSUM
                                # bank -> split the 768-col V output at 512
                                for v0, v1 in ((0, 512), (512, C)):
                                    for f in range(FT):
                                        nc.tensor.matmul(
                                            ps[:].rearrange(
                                                "p h s -> p (h s)")[:, v0:v1],
                                            xf[:, f, c0 : c0 + P],
                                            wqkv_t[:, f, 2 * C + v0 : 2 * C + v1],
                                            start=(f == 0), stop=(f == FT - 1),
                                        )
                                nc.vector.tensor_copy(
                                    v_sb[:, vb, :, 0:HS], ps[:])

                        def attn_block(ql):
                            """Attention for owned query block ql (0=early)."""
                            W = QW[ql]
                            moff = 0 if ql == 0 else 4 * P
                            for h in range(H):
                                hp, f = HS * (h % 2), h // 2
                                s_ps = pp.tile([P, W * P], F32, tag="s", bufs=2)
                                for kb in range(W):
                                    nc.tensor.matmul(
                                        s_ps[:, kb * P : (kb + 1) * P],
                                        kf[hp : hp + HS, f,
                                           COL[kb] : COL[kb] + P],
                                        q_sb[hp : hp + HS, f,
                                             ql * P : (ql + 1) * P],
                                        start=True, stop=True,
                                    )
                                pt = ap.tile([P, W * P], BF16,
                                             tag="pt", bufs=2, name="pt")
                                nc.scalar.activation(pt[:], s_ps[:], AF.Exp)
                                # q1's kv blocks 0..3 are always fully causal
                                # (query block >= 4): only mask the rest
                                mq0 = 0 if ql == 0 else 4 * P
                                nc.vector.tensor_tensor(
                                    pt[:, mq0 : W * P], pt[:, mq0 : W * P],
                                    msk[:, 0 if ql == 0 else 4 * P : 8 * P
                                        if ql else 4 * P], op=OP.mult)
                                ov = pp.tile([P, TB], F32, tag="o", bufs=2)
                                for kb in range(W):
                                    nc.tensor.matmul(
                                        ov[: HS + 1, 0:P],
                                        v_sb[:, COL[kb] // P, h, :],
                                        pt[:, kb * P : (kb + 1) * P],
                                        start=(kb == 0), stop=(kb == W - 1),
                                    )
                                rden = rp.tile([1, P], BF16, tag="rden",
                                               bufs=2)
                                with nc.allow_low_precision(
                                        reason="softmax rden bf16"):
                                    nc.vector.reciprocal(rden[:1, :],
                                                         ov[HS : HS + 1, 0:P])
                                nc.tensor.matmul(
                                    ov[0:HS, P : P + P],
                                    ones_row_bf[:1, 0:HS], rden[:1, :],
                                    start=True, stop=True,
                                )
                                sc_sb = ap.tile([HS, P], BF16, tag="scsb",
                                                bufs=2)
                                nc.scalar.copy(sc_sb[:], ov[0:HS, P : P + P])
                                nc.vector.tensor_tensor(
                                    o_fm[hp : hp + HS, f,
                                         ql * P : (ql + 1) * P],
                                    ov[0:HS, 0:P], sc_sb[:],
                                    op=OP.mult,
                                )

                        if attn:
                            # scheduler-only fences: keep the collective-
                            # gated KV work from head-of-line blocking the
                            # in-order engine streams ahead of ready work
                            tc.no_sync_barrier()
                            kv_half(0)
                            attn_block(0)
                            tc.no_sync_barrier()
                            kv_half(1)
                            attn_block(1)
                        else:
                            for f in range(FT):
                                nc.vector.tensor_copy(o_fm[:, f, :],
                                                      xh[:, f, :])

                        # remaining weights on the Act HWDGE queue so the SP
                        # queue stays clear for collective/xf staging
                        wp_t = wp_pool.tile([P, FT, C], BF16, tag="wp")
                        nc.scalar.dma_start(
                            wp_t[:], wp_d.ap()[l].rearrange("f p m -> p f m"))
                        w1c = []
                        for ch in range(4):
                            w1ct = wp_pool.tile([P, FT, C], BF16, tag="w1",
                                                bufs=2, name=f"w1c{ch}")
                            w1c.append(w1ct)
                            nc.scalar.dma_start(
                                w1ct[:],
                                w1_d.ap()[l, :, :, ch * C : (ch + 1) * C]
                                .rearrange("f p m -> p f m"))
                        w2c = []
                        for ch in range(2):
                            w2ct = wp_pool.tile([P, F4, 3 * P], BF16,
                                                tag="w2", bufs=2,
                                                name=f"w2c{ch}")
                            w2c.append(w2ct)
                            nc.scalar.dma_start(
                                w2ct[:],
                                w2_d.ap()[l, :, :, ch * 3 * P : (ch + 1) * 3 * P]
                                .rearrange("f p m -> p f m"))

                        # -- output projection + residual --
                        x_new = ap.tile([P, FT, TB], BF16, tag="x", bufs=2)
                        for o in range(FT):
                            ps = pp.tile([P, TB], F32, tag="g", bufs=2)
                            for f in range(FT):
                                nc.tensor.matmul(
                                    ps[:], wp_t[:, f, o * P : (o + 1) * P],
                                    o_fm[:, f, :], start=(f == 0),
                                    stop=(f == FT - 1),
                                )
                            nc.vector.scalar_tensor_tensor(
                                x_new[:, o, :], in0=ps[:],
                                scalar=bpc[:, o : o + 1],
                                in1=x_cur[:, o, :], op0=OP.add, op1=OP.add,
                            )
                        x_cur = x_new

                        # -- FFN --
                        xh2 = layernorm(2 * l + 1, x_cur, out_tag="ofm")
                        h1 = ap.tile([P, F4, TB], BF16, tag="h1")
                        for o in range(F4):
                            ps = pp.tile([P, TB], F32, tag="g", bufs=2)
                            for f in range(FT):
                                nc.tensor.matmul(
                                    ps[:],
                                    w1c[o // FT][:, f, (o % FT) * P : (o % FT + 1) * P],
                                    xh2[:, f, :], start=(f == 0),
                                    stop=(f == FT - 1),
                                )
                            nc.scalar.activation(h1[:, o, :], ps[:], AF.Relu,
                                                 bias=b1c[:, o : o + 1])
                        x_new = ap.tile([P, FT, TB], BF16, tag="x", bufs=2)
                        for o in range(FT):
                            ps = pp.tile([P, TB], F32, tag="g", bufs=2)
                            for f in range(F4):
                                nc.tensor.matmul(
                                    ps[:],
                                    w2c[o // 3][:, f, (o % 3) * P : (o % 3 + 1) * P],
                                    h1[:, f, :], start=(f == 0),
                                    stop=(f == F4 - 1),
                                )
                            nc.vector.scalar_tensor_tensor(
                                x_new[:, o, :], in0=ps[:],
                                scalar=b2c[:, o : o + 1],
                                in1=x_cur[:, o, :], op0=OP.add, op1=OP.add,
                            )
                        x_cur = x_new

                    xhf = layernorm(2 * L, x_cur)

            # ---- LM head (token-parallel over own 256 tokens) ----
            if head:
                with (
                    tc.tile_pool(name="hpsum", bufs=1, space="PSUM") as hpp,
                    tc.tile_pool(name="head", bufs=1) as hp_pool,
                ):
                    for vb0 in range(0, V, VC2):
                        vw_c = min(VC2, V - vb0)  # last chunk is 256 wide
                        wh_t = hp_pool.tile([P, FT, VC2], BF16, tag="wh",
                                            bufs=2)
                        nc.scalar.dma_start(
                            wh_t[:, :, 0:vw_c],
                            wh_d.ap()[:, :, vb0 : vb0 + vw_c]
                            .rearrange("f p m -> p f m"),
                        )
                        for tt in range(2):
                            ps = hpp.tile([P, VC2], F32, tag="h", bufs=4)
                            for v0 in range(0, vw_c, 512):
                                vw = min(512, vw_c - v0)
                                for f in range(FT):
                                    nc.tensor.matmul(
                                        ps[:, v0 : v0 + vw],
                                        xhf[:, f, tt * P : (tt + 1) * P],
                                        wh_t[:, f, v0 : v0 + vw],
                                        start=(f == 0), stop=(f == FT - 1),
                                    )
                            ob = hp_pool.tile([P, VC2], BF16, tag="ob", bufs=4)
                            nc.scalar.copy(ob[:, 0:vw_c], ps[:, 0:vw_c])
                            nc.sync.dma_start(
                                out_d.ap()[tt, :, vb0 : vb0 + vw_c],
                                ob[:, 0:vw_c],
                            )

    nc.compile()
    return nc


def prep_inputs(inputs):
    """Host-side sharding: returns in_maps (one dict per core)."""
    bf = ml_dtypes.bfloat16
    g = {k: np.asarray(v) for k, v in inputs.items()}
    idx = g["idx"].astype(np.int64)
    tok = np.asarray(g["tok_emb"], np.float32)
    pos = np.asarray(g["pos_emb"], np.float32)

    def fm(w):  # [C_in, M] -> [FT, P, M] bf16
        return np.ascontiguousarray(w.reshape(FT, P, -1)).astype(bf)

    wqkv = np.empty((L, FT, P, 3 * C), bf)
    wp_a = np.empty((L, FT, P, C), bf)
    w1_a = np.empty((L, FT, P, 4 * C), bf)
    w2_a = np.empty((L, F4, P, C), bf)
    for l in range(L):
        q = np.transpose(np.asarray(g["Wq"][l], np.float32), (1, 0, 2)).reshape(C, C)
        k = np.transpose(np.asarray(g["Wk"][l], np.float32), (1, 0, 2)).reshape(C, C)
        v = np.transpose(np.asarray(g["Wv"][l], np.float32), (1, 0, 2)).reshape(C, C)
        wqkv[l] = fm(np.concatenate([q * SCALE, k, v], axis=1))
        wp_a[l] = fm(np.asarray(g["Wp"][l], np.float32))
        w1_a[l] = fm(np.asarray(g["W1"][l], np.float32))
        w2_a[l] = np.asarray(g["W2"][l], np.float32).reshape(F4, P, C).astype(bf)

    lng = np.stack(
        [np.asarray(g["ln1g"][l // 2] if l % 2 == 0 else g["ln2g"][l // 2],
                    np.float32)
         for l in range(2 * L)] + [np.asarray(g["lnfg"], np.float32)]
    ).astype(bf)
    lnb = np.stack(
        [np.asarray(g["ln1b"][l // 2] if l % 2 == 0 else g["ln2b"][l // 2],
                    np.float32)
         for l in range(2 * L)] + [np.asarray(g["lnfb"], np.float32)]
    )

    wh_full = np.asarray(g["Wh"], np.float32).reshape(FT, P, V).astype(bf)

    # per-rank causal masks in S^T ([key, query]) layout, kv blocks in global
    # order: early query block uses kv blocks 0..3, late uses 0..7.
    tri = (np.arange(P)[:, None] <= np.arange(P)[None, :]).astype(np.float32)

    in_maps = []
    for r in range(NCORES):
        bt = r // 4
        lr = r % 4
        blocks = _blocks_of(r)
        e = np.concatenate(
            [tok[idx[bt, gb * P : (gb + 1) * P]] + pos[gb * P : (gb + 1) * P]
             for gb in blocks], axis=0)  # [TB, C]
        x0 = np.ascontiguousarray(
            e.T.reshape(FT, P, TB).transpose(1, 0, 2)).astype(bf)

        m = np.zeros((P, 8 * P), np.float32)
        for ql, gq in enumerate(blocks):
            kbs = range(0, 4) if ql == 0 else range(4, 8)
            for j, kb in enumerate(kbs):
                blk = m[:, (0 if ql == 0 else 4 * P) + j * P :][:, :P]
                if kb < gq:
                    blk[:] = 1.0
                elif kb == gq:
                    blk[:] = tri

        in_maps.append({
            "x0": x0,
            "wqkv": wqkv, "wp": wp_a, "w1": w1_a, "w2": w2_a,
            "wh": wh_full,
            "lng": lng, "lnb": lnb,
            "bp": np.asarray(g["bp"], np.float32),
            "b1": np.asarray(g["b1"], np.float32),
            "b2": np.asarray(g["b2"], np.float32),
            "msk": m.astype(bf),
        })
    return in_maps


_CACHED_NC = None


def kernel(**inputs):
    global _CACHED_NC
    if _CACHED_NC is None:
        _CACHED_NC = build()
    nc = _CACHED_NC
    in_maps = prep_inputs(inputs)
    res = run_bass_kernel_spmd(nc, in_maps, core_ids=list(range(NCORES)))
    logits = np.empty((B, T, V), np.float32)
    for r in range(NCORES):
        bt = r // 4
        out = np.asarray(res.results[r]["out"], np.float32)
        for i, gb in enumerate(_blocks_of(r)):
            logits[bt, gb * P : (gb + 1) * P, :] = out[i]
    return logits

